# revision 1
# baseline (speedup 1.0000x reference)
"""Trainium2 Bass kernel for nn_GCNN_87668872446200.

Two GCNConv+pool protein branches + two masif conv branches + dense head,
distributed over 8 NeuronCores as 4 feature-slices x 2 dest-node halves.

Per core:
  - xw = x @ W[:, fslice]  (PE matmul from host-pretransposed xT, bf16)
  - xw written to HBM; dma_gather pulls source rows for this core's edge half
    (512B descriptors, full DMA rate)
  - scatter-add realized as PE matmuls: S[128 edges, 64 dests] (host-built,
    norm-scaled, bf16, streamed from HBM) x gathered[128 edges, 256 feats]
    accumulated in PSUM per 64-dest block
  - +bias (DVE), LeakyReLU (ACT) -> h block [64, 256] bf16
  - per-graph mean-pool as PE matmul with host-built Mpool (folds 1/cnt)
  - masif branch: 4 graphs/core, reduces+ACT+tiny matmuls
  - one AllReduce (~272KB) merges pooled features + masif outputs
  - replicated dense head -> sigmoid -> [1, 32] output (core 0's is used)

All 8 cores run ONE identical program; per-core variation is in input data
(weight slices, gather indices, S/Mpool matrices, masks).
"""
import math
import numpy as np

# ---------------------------------------------------------------- constants
N_CORES = 8
N_FSLICE = 2      # feature slices (F // N_FSLICE features per core)
N_DPART = 4       # destination-node partitions
P = 128
BLK = 64          # dest nodes per scatter block (S width)
GRP = 8           # chunks per gather/S group (1024 idxs per dma_gather;
                  # HW fails somewhere in (1024, 2048] idxs per call)

# problem sizes (hardcoded per spec)
N_NODES, N_EDGES, F_DIM, B_GRAPHS, L_MAS, C_MAS = 10000, 80000, 1024, 32, 800, 16


class _Cfg:
    def __init__(self, n=N_NODES, e=N_EDGES, f=F_DIM, b=B_GRAPHS,
                 l=L_MAS, c=C_MAS):
        assert f % 512 == 0 and b == 32 and l % 80 == 0 and c % 2 == 0
        self.N, self.E, self.F, self.B, self.L, self.C = n, e, f, b, l, c
        self.NPAD = ((n + 511) // 512) * 512
        while (self.NPAD // N_DPART) % BLK or (self.NPAD % 512):
            self.NPAD += 512
        self.HALF = self.NPAD // N_DPART       # nodes per dest partition
        self.NBLK = self.HALF // BLK           # blocks per dest partition
        self.FSL = f // N_FSLICE               # features per core slice
        self.KC = f // P                       # k-chunks of contraction
        self.GPB = b // N_CORES                # graphs per core for masif
        self.LW = l // 80                      # avg-pool window (10)
        self.LB = 8                            # l-blocks for masif layout
        self.LBS = l // self.LB                # l-block size (100)
        assert self.LBS % self.LW == 0
        self.WPB = self.LBS // self.LW         # windows per l-block (10)


# ---------------------------------------------------------------- host prep
def _edge_plan(cfg, edge_index):
    """Per-half sorted/chunked scatter plans with a shared per-block chunk
    schedule (max over halves), padded to a multiple of GRP chunks."""
    row = np.asarray(edge_index[0]).astype(np.int64)
    col = np.asarray(edge_index[1]).astype(np.int64)
    loops = np.arange(cfg.N, dtype=np.int64)
    rows = np.concatenate([row, loops])
    cols = np.concatenate([col, loops])
    deg = np.bincount(cols, minlength=cfg.N).astype(np.float64)
    dinv = 1.0 / np.sqrt(deg)
    norm = (dinv[rows] * dinv[cols]).astype(np.float32)

    halves = []
    counts = np.zeros((N_DPART, cfg.NBLK), np.int64)
    for hf in range(N_DPART):
        lo, hi = hf * cfg.HALF, (hf + 1) * cfg.HALF
        sel = (cols >= lo) & (cols < hi)
        r, c, w = rows[sel], cols[sel], norm[sel]
        order = np.argsort(c, kind='stable')
        r, c, w = r[order], c[order], w[order]
        blk = (c - lo) // BLK
        starts = np.searchsorted(blk, np.arange(cfg.NBLK), side='left')
        ends = np.searchsorted(blk, np.arange(cfg.NBLK), side='right')
        counts[hf] = np.maximum((ends - starts + 127) // 128, 1)
        halves.append((r, c - lo, w, starts, ends))

    kj = counts.max(0)                       # shared chunks per block
    c_total = int(kj.sum())
    c_pad = ((c_total + GRP - 1) // GRP) * GRP
    kj[-1] += c_pad - c_total                # tail dummies on last block

    # shared schedule: list of (block, start_flag, stop_flag)
    sched = []
    for j in range(cfg.NBLK):
        for k in range(kj[j]):
            sched.append((j, k == 0, k == kj[j] - 1))
    assert len(sched) == c_pad

    # per-part streams
    srcs_all, s_all = [], []
    for hf in range(N_DPART):
        r, cl, w, starts, ends = halves[hf]
        srcs = np.zeros((c_pad, P), np.int16)
        smat = np.zeros((c_pad, P, BLK), np.float32)
        i = 0
        for j in range(cfg.NBLK):
            s0, e0 = starts[j], ends[j]
            for k in range(kj[j]):
                cs = s0 + k * P
                ce = min(cs + P, e0)
                if ce > cs:
                    n = ce - cs
                    srcs[i, :n] = r[cs:ce]
                    smat[i, np.arange(n), cl[cs:ce] - j * BLK] = w[cs:ce]
                i += 1
        srcs_all.append(srcs)
        s_all.append(smat)
    return sched, c_pad, srcs_all, s_all


def _wrap_idxs(srcs):
    """[C, 128] int16 -> wrapped [128, C*8] (idx j at [j%16 + 16*rep, j//16])."""
    flat = srcs.reshape(-1)
    w = flat.reshape(-1, 16).T                # [16, C*8]
    return np.tile(w, (8, 1)).astype(np.int16)


def _group_s(smat, dt):
    """[C, 128, 64] -> [C//GRP, 128, GRP*64] grouped for contiguous loads."""
    c = smat.shape[0]
    g = smat.reshape(c // GRP, GRP, P, BLK).transpose(0, 2, 1, 3)
    return np.ascontiguousarray(g.reshape(c // GRP, P, GRP * BLK)).astype(dt)


def _mpool(cfg, batch, hf, dt):
    """[HALF, B] matrix folding 1/cnt, zero rows for pad nodes."""
    batch = np.asarray(batch).astype(np.int64)
    cnt = np.bincount(batch, minlength=cfg.B).astype(np.float64)
    cinv = 1.0 / np.maximum(cnt, 1.0)
    m = np.zeros((cfg.HALF, cfg.B), np.float32)
    lo = hf * cfg.HALF
    hi = min(lo + cfg.HALF, cfg.N)
    if hi > lo:
        rows = np.arange(lo, hi)
        m[rows - lo, batch[rows]] = cinv[batch[rows]].astype(np.float32)
    return m.astype(dt)


def _preprocess(inputs, cfg, mm_dt, gs_dt):
    """Build shared program meta + per-core input maps (numpy host work)."""
    f32, bf16 = np.float32, gs_dt
    meta = {}
    shared = {}

    # pretransposed, padded, cast x
    for br in (1, 2):
        x = np.asarray(inputs[f'pro{br}_x'], np.float32)
        xt = np.zeros((cfg.F, cfg.NPAD), mm_dt)
        xt[:, :cfg.N] = x.T.astype(mm_dt)
        shared[f'xT{br}'] = xt
        sched, c_pad, srcs, smat = _edge_plan(cfg, inputs[f'pro{br}_edge_index'])
        meta[f'sched{br}'] = sched
        meta[f'cpad{br}'] = c_pad
        shared[f'_srcs{br}'] = srcs
        shared[f'_smat{br}'] = smat

    # head weights (shared, bf16 for matmuls / f32 biases as [*, 1])
    def colv(v, n):
        return np.asarray(v, np.float32).reshape(n, 1)
    shared['W_pf1'] = np.asarray(inputs['W_pf1'], np.float32)
    shared['W_pf2'] = np.asarray(inputs['W_pf2'], np.float32)
    shared['W_fc1'] = np.asarray(inputs['W_fc1'], np.float32)
    shared['W_fc2'] = np.asarray(inputs['W_fc2'], np.float32)
    wo = np.zeros((256, 1), np.float32)
    wo[0:64] = np.asarray(inputs['W_out'], np.float32)[0:64]
    wo[128:256] = np.asarray(inputs['W_out'], np.float32)[64:192]
    shared['W_out'] = wo
    shared['b_pf1'] = colv(inputs['b_pf1'], 128)
    shared['b_pf2'] = colv(inputs['b_pf2'], 128)
    shared['b_fc1'] = colv(inputs['b_fc1'], 256)
    shared['b_fc2'] = colv(inputs['b_fc2'], 64)
    shared['b_out'] = colv(inputs['b_out'], 1)
    for m in (1, 2):
        shared[f'W_m{m}'] = (np.asarray(inputs[f'W_m{m}'], np.float32)
                             / (2.0 * cfg.LW)).reshape(8, 10, 64)
        shared[f'b_m{m}'] = colv(inputs[f'b_m{m}'], 64)
        for sf, pre in (('s', 'cs'), ('f', 'cf')):
            w = float(np.asarray(inputs[f'{pre}{m}_w'])[0])
            b = float(np.asarray(inputs[f'{pre}{m}_b'])[0])
            shared[f'scale_{sf}{m}'] = np.full((32, 1), w / cfg.C, np.float32)
            shared[f'bias_{sf}{m}'] = np.full((32, 1), b, np.float32)

    in_maps = []
    for core in range(N_CORES):
        fs, hf = core % N_FSLICE, core // N_FSLICE
        f_lo = fs * cfg.FSL
        m = {k: v for k, v in shared.items() if not k.startswith('_')}
        for br in (1, 2):
            W = np.asarray(inputs[f'W_g{br}'], np.float32)[:, f_lo:f_lo + cfg.FSL]
            m[f'Wg{br}'] = np.ascontiguousarray(
                W.reshape(cfg.KC, P, cfg.FSL)).astype(mm_dt)
            bia = np.asarray(inputs[f'b_g{br}'], np.float32)[f_lo:f_lo + cfg.FSL]
            m[f'bg{br}'] = np.tile(bia[None, :], (P, 1)).astype(np.float32)
            m[f'idx{br}'] = _wrap_idxs(shared[f'_srcs{br}'][hf])
            m[f'smat{br}'] = _group_s(shared[f'_smat{br}'][hf], gs_dt)
            m[f'mpool{br}'] = _mpool(cfg, inputs[f'pro{br}_batch'], hf, gs_dt)
        # pooled placement mask [B, N_FSLICE*FSL]
        pm = np.zeros((cfg.B, cfg.F), np.float32)
        pm[:, f_lo:f_lo + cfg.FSL] = 1.0
        m['fmask'] = pm
        # masif slices + placement mask
        gsel = slice(core * cfg.GPB, (core + 1) * cfg.GPB)
        for mi, names in ((1, ('mas1_straight', 'mas1_flipped')),
                          (2, ('mas2_straight', 'mas2_flipped'))):
            m[f'mas{mi}s'] = np.ascontiguousarray(
                np.asarray(inputs[names[0]], np.float32)[gsel])
            m[f'mas{mi}f'] = np.ascontiguousarray(
                np.asarray(inputs[names[1]], np.float32)[gsel])
        mk = np.zeros((P, cfg.B), np.float32)
        mk[:, core * cfg.GPB:(core + 1) * cfg.GPB] = 1.0
        m['gmask'] = mk
        in_maps.append(m)
    return meta, in_maps


# ---------------------------------------------------------------- program
def _build(cfg, meta, mm_dt_np, gs_dt_np):
    import concourse.bass as bass
    import concourse.bacc as bacc
    import concourse.mybir as mybir
    import concourse.tile as tile
    from concourse.masks import make_identity

    dt = mybir.dt
    mm_dt = dt.from_np(np.dtype(mm_dt_np))
    gs_dt = dt.from_np(np.dtype(gs_dt_np))
    f32 = dt.float32
    AF = mybir.ActivationFunctionType
    OP = mybir.AluOpType

    nc = bacc.Bacc("TRN2", target_bir_lowering=False, debug=False,
                   enable_asserts=False, num_devices=N_CORES)

    # ---- dram tensors (inputs)
    def din(name, shape, d):
        return nc.dram_tensor(name, list(shape), d, kind="ExternalInput")

    xT = {br: din(f'xT{br}', (cfg.F, cfg.NPAD), mm_dt) for br in (1, 2)}
    Wg = {br: din(f'Wg{br}', (cfg.KC, P, cfg.FSL), mm_dt) for br in (1, 2)}
    bg = {br: din(f'bg{br}', (P, cfg.FSL), f32) for br in (1, 2)}
    idx = {br: din(f'idx{br}', (P, meta[f'cpad{br}'] * 8), dt.int16)
           for br in (1, 2)}
    smat = {br: din(f'smat{br}', (meta[f'cpad{br}'] // GRP, P, GRP * BLK), gs_dt)
            for br in (1, 2)}
    mpool = {br: din(f'mpool{br}', (cfg.HALF, cfg.B), gs_dt) for br in (1, 2)}
    fmask = din('fmask', (cfg.B, cfg.F), f32)
    gmask = din('gmask', (P, cfg.B), f32)
    mas = {(mi, sf): din(f'mas{mi}{sf}', (cfg.GPB, cfg.C, cfg.L), f32)
           for mi in (1, 2) for sf in 'sf'}
    w_pf = {1: din('W_pf1', (cfg.F, P), f32), 2: din('W_pf2', (cfg.F, P), f32)}
    b_pf = {1: din('b_pf1', (P, 1), f32), 2: din('b_pf2', (P, 1), f32)}
    w_fc1 = din('W_fc1', (256, 256), f32)
    w_fc2 = din('W_fc2', (256, 64), f32)
    b_fc1 = din('b_fc1', (256, 1), f32)
    b_fc2 = din('b_fc2', (64, 1), f32)
    w_out = din('W_out', (256, 1), f32)
    b_out = din('b_out', (1, 1), f32)
    w_m = {mi: din(f'W_m{mi}', (8, 10, 64), f32) for mi in (1, 2)}
    b_m = {mi: din(f'b_m{mi}', (64, 1), f32) for mi in (1, 2)}
    msc = {(mi, sf, kind): din(f'{kind}_{sf}{mi}', (32, 1), f32)
           for mi in (1, 2) for sf in 'sf' for kind in ('scale', 'bias')}

    out_t = nc.dram_tensor('out', [1, cfg.B], f32, kind="ExternalOutput")

    ASM = cfg.B * 2 * cfg.F + P * cfg.B   # allreduce payload (f32 elements)

    with tile.TileContext(nc) as tc:
        with tc.tile_pool(name="const", bufs=1) as cst, \
             tc.tile_pool(name="xt", bufs=2) as xtp, \
             tc.tile_pool(name="xwps", bufs=2, space="PSUM") as xwps, \
             tc.tile_pool(name="xwsb", bufs=3) as xwsb, \
             tc.tile_pool(name="gat", bufs=3) as gatp, \
             tc.tile_pool(name="sld", bufs=3) as sldp, \
             tc.tile_pool(name="idxp", bufs=3) as idxp, \
             tc.tile_pool(name="blkps", bufs=2, space="PSUM") as blkps, \
             tc.tile_pool(name="h블", bufs=4) as hp, \
             tc.tile_pool(name="poolps", bufs=2, space="PSUM") as poolps, \
             tc.tile_pool(name="small", bufs=2) as smp, \
             tc.tile_pool(name="smallps", bufs=1, space="PSUM") as smps, \
             tc.tile_pool(name="dram", bufs=2, space="DRAM") as drp:

            # ---------------- constants into SBUF
            def load(pool, src_ap, shape, d, name=None):
                t = pool.tile(list(shape), d, tag=name)
                nc.sync.dma_start(out=t[:], in_=src_ap)
                return t

            wg_sb = {br: load(cst, Wg[br].ap().transpose([1, 0, 2]),
                              (P, cfg.KC, cfg.FSL), mm_dt, f'wg{br}')
                     for br in (1, 2)}
            bg_sb = {br: load(cst, bg[br][:, :], (P, cfg.FSL), f32, f'bg{br}')
                     for br in (1, 2)}
            mp_sb = {br: load(cst,
                              mpool[br].ap().rearrange(
                                  "(j d) g -> j d g", d=BLK).transpose([1, 0, 2]),
                              (BLK, cfg.NBLK, cfg.B), gs_dt, f'mp{br}')
                     for br in (1, 2)}
            fmask_sb = load(cst, fmask[:, :], (cfg.B, cfg.F), f32, 'fmask')
            gmask_sb = load(cst, gmask[:, :], (P, cfg.B), f32, 'gmask')
            id32 = cst.tile([32, 32], f32, tag='id32')
            make_identity(nc, id32[:])

            wpf_sb = {br: load(cst, w_pf[br].ap().rearrange(
                "(k p) m -> k p m", p=P).transpose([1, 0, 2]),
                               (P, cfg.KC, P), f32, f'wpf{br}') for br in (1, 2)}
            bpf_sb = {br: load(cst, b_pf[br][:, :], (P, 1), f32, f'bpf{br}')
                      for br in (1, 2)}
            wfc1_sb = load(cst, w_fc1.ap().rearrange(
                "(a p) m -> a p m", p=P).transpose([1, 0, 2]),
                           (P, 2, 256), f32, 'wfc1')
            wfc2_sb = load(cst, w_fc2.ap().rearrange(
                "(a p) m -> a p m", p=P).transpose([1, 0, 2]),
                           (P, 2, 64), f32, 'wfc2')
            bfc1_sb = load(cst, b_fc1.ap().rearrange(
                "(a p) m -> a p m", p=P).transpose([1, 0, 2]),
                           (P, 2, 1), f32, 'bfc1')
            bfc2_sb = load(cst, b_fc2[:, :], (64, 1), f32, 'bfc2')
            wout_sb = load(cst, w_out.ap().rearrange(
                "(a p) m -> a p m", p=P).transpose([1, 0, 2]),
                           (P, 2, 1), f32, 'wout')
            bout_sb = load(cst, b_out[:, :], (1, 1), f32, 'bout')
            wm_sb = {mi: load(cst, w_m[mi].ap().transpose([1, 0, 2]),
                              (10, 8, 64), f32, f'wm{mi}') for mi in (1, 2)}
            bm_sb = {mi: load(cst, b_m[mi][:, :], (64, 1), f32, f'bm{mi}')
                     for mi in (1, 2)}
            msc_sb = {k: load(cst, v[:, :], (32, 1), f32, f'msc{k}')
                      for k, v in msc.items()}

            # ---------------- masif (both branches) -> two [64, B] f32 tiles
            masif_asm1 = cst.tile([64, cfg.B], f32, tag='masasm1')
            masif_asm2 = cst.tile([64, cfg.B], f32, tag='masasm2')
            masif_asm = {1: masif_asm1, 2: masif_asm2}
            for mi in (1, 2):
                frag = None
                for sf in 'sf':
                    src = mas[(mi, sf)]
                    t = smp.tile([32, cfg.C, cfg.LBS], f32, tag='masload')
                    for lb in range(cfg.LB):
                        nc.sync.dma_start(
                            out=t[lb * cfg.GPB:(lb + 1) * cfg.GPB],
                            in_=src.ap()[:, :, lb * cfg.LBS:(lb + 1) * cfg.LBS])
                    red = smp.tile([32, cfg.LBS], f32, tag='masred')
                    nc.vector.tensor_reduce(
                        out=red[:], in_=t[:].transpose([0, 2, 1]),
                        axis=mybir.AxisListType.X, op=OP.add)
                    act = smp.tile([32, cfg.LBS], f32, tag='masact')
                    nc.scalar.activation(
                        act[:], red[:], AF.Relu,
                        bias=msc_sb[(mi, sf, 'bias')][:, 0:1],
                        scale=msc_sb[(mi, sf, 'scale')][:, 0:1])
                    ws = smp.tile([32, cfg.WPB], f32, tag='masws')
                    nc.vector.tensor_reduce(
                        out=ws[:],
                        in_=act[:].rearrange("p (w l) -> p w l", l=cfg.LW),
                        axis=mybir.AxisListType.X, op=OP.add)
                    if frag is None:
                        frag = ws
                    else:
                        frag2 = smp.tile([32, cfg.WPB], f32, tag='masfrag')
                        nc.vector.tensor_add(out=frag2[:], in0=frag[:], in1=ws[:])
                        frag = frag2
                # transpose [32, 10] -> [10, 32]
                ps_t = smps.tile([cfg.WPB, 32], f32, space="PSUM", tag='sps')
                nc.tensor.transpose(out=ps_t[:], in_=frag[:], identity=id32[:])
                fragT = smp.tile([cfg.WPB, 32], f32, tag='masfragT')
                nc.scalar.activation(fragT[:], ps_t[:], AF.Identity)
                fragTc = fragT[:].rearrange("k (lb g) -> k lb g", g=cfg.GPB)
                m_ps = smps.tile([64, cfg.GPB], f32, space="PSUM", tag='spsacc')
                for lb in range(cfg.LB):
                    nc.tensor.matmul(
                        m_ps[:], lhsT=wm_sb[mi][:, lb, :], rhs=fragTc[:, lb, :],
                        start=(lb == 0), stop=(lb == cfg.LB - 1))
                m_fm = smp.tile([64, cfg.GPB], f32, tag='masfm')
                nc.scalar.activation(m_fm[:], m_ps[:], AF.Identity,
                                     bias=bm_sb[mi][:, 0:1])
                # broadcast to [64, B] * gmask -> masif_asm[mi]
                nc.vector.tensor_tensor(
                    out=masif_asm[mi][:].rearrange(
                        "p (s g) -> p s g", g=cfg.GPB),
                    in0=m_fm[:, None, :].to_broadcast(
                        [64, N_CORES, cfg.GPB]),
                    in1=gmask_sb[0:64, :].rearrange(
                        "p (s g) -> p s g", g=cfg.GPB),
                    op=OP.mult)

            # ---------------- GCN branches
            pooled_full = cst.tile([cfg.B, 2 * cfg.F], f32, tag='poolfull')
            NT = cfg.NPAD // 512
            for br in (1, 2):
                cpad = meta[f'cpad{br}']
                sched = meta[f'sched{br}']
                xw_dram = drp.tile([cfg.NPAD, cfg.FSL], gs_dt, tag='xwdram')
                # xw = x @ W slice
                for nt in range(NT):
                    xt_t = xtp.tile([P, cfg.KC, 512], mm_dt, tag='xt')
                    for k in range(cfg.KC):
                        nc.sync.dma_start(
                            out=xt_t[:, k, :],
                            in_=xT[br][k * P:(k + 1) * P,
                                       nt * 512:(nt + 1) * 512])
                    for sub in range(4):
                        ps = xwps.tile([P, cfg.FSL], f32, space="PSUM",
                                       tag='xwps')
                        for k in range(cfg.KC):
                            nc.tensor.matmul(
                                ps[:],
                                lhsT=xt_t[:, k, sub * P:(sub + 1) * P],
                                rhs=wg_sb[br][:, k, :],
                                start=(k == 0), stop=(k == cfg.KC - 1))
                        xw_t = xwsb.tile([P, cfg.FSL], gs_dt, tag='xwsb')
                        nc.scalar.activation(xw_t[:], ps[:], AF.Identity)
                        nc.sync.dma_start(
                            out=xw_dram[(nt * 4 + sub) * P:
                                        (nt * 4 + sub + 1) * P, :],
                            in_=xw_t[:])

                # scatter + pool
                pool_ps = poolps.tile([cfg.B, cfg.FSL], f32, space="PSUM",
                                      tag='poolps')
                n_grp = cpad // GRP
                ci = 0
                blk_ps = None
                for g in range(n_grp):
                    idx_t = idxp.tile([P, GRP * 8], dt.int16, tag='idx')
                    nc.sync.dma_start(
                        out=idx_t[:],
                        in_=idx[br][:, g * GRP * 8:(g + 1) * GRP * 8])
                    gat_t = gatp.tile([P, GRP, cfg.FSL], gs_dt, tag='gat')
                    nc.gpsimd.dma_gather(
                        out_ap=gat_t[:], in_ap=xw_dram[:, :], idxs_ap=idx_t[:],
                        num_idxs=GRP * P, num_idxs_reg=GRP * P,
                        elem_size=cfg.FSL)
                    s_t = sldp.tile([P, GRP * BLK], gs_dt, tag='sld')
                    nc.sync.dma_start(out=s_t[:], in_=smat[br][g, :, :])
                    for i in range(GRP):
                        j, st, sp = sched[ci]
                        if st:
                            blk_ps = blkps.tile([BLK, cfg.FSL], f32,
                                                space="PSUM", tag='blkps')
                        nc.tensor.matmul(
                            blk_ps[:],
                            lhsT=s_t[:, i * BLK:(i + 1) * BLK],
                            rhs=gat_t[:, i, :],
                            start=st, stop=sp)
                        if sp:
                            h_t = hp.tile([BLK, cfg.FSL], gs_dt, tag='h')
                            nc.vector.tensor_add(out=h_t[:], in0=blk_ps[:],
                                                 in1=bg_sb[br][0:BLK, :])
                            nc.scalar.activation(h_t[:], h_t[:], AF.Lrelu,
                                                 alpha=0.01)
                            nc.tensor.matmul(
                                pool_ps[:], lhsT=mp_sb[br][:, j, :],
                                rhs=h_t[:],
                                start=(j == 0), stop=(j == cfg.NBLK - 1))
                        ci += 1
                # pooled [B, FSL] -> mask-place into pooled_full
                pooled_sb = smp.tile([cfg.B, cfg.FSL], f32, tag='pooled')
                nc.scalar.activation(pooled_sb[:], pool_ps[:], AF.Identity)
                nc.vector.tensor_tensor(
                    out=pooled_full[:, (br - 1) * cfg.F:br * cfg.F].rearrange(
                        "p (s m) -> p s m", m=cfg.FSL),
                    in0=pooled_sb[:, None, :].to_broadcast(
                        [cfg.B, N_FSLICE, cfg.FSL]),
                    in1=fmask_sb[:, :].rearrange("p (s m) -> p s m", m=cfg.FSL),
                    op=OP.mult)

            # ---------------- allreduce
            bounce_in = drp.tile([ASM], f32, tag='ccin')
            bounce_out = drp.tile([ASM], f32, tag='ccout')
            pf_n = cfg.B * 2 * cfg.F
            nc.sync.dma_start(
                out=bounce_in[0:pf_n].rearrange("(p f) -> p f", f=2 * cfg.F),
                in_=pooled_full[:])
            half_m = 64 * cfg.B
            for mi in (1, 2):
                lo = pf_n + (mi - 1) * half_m
                nc.sync.dma_start(
                    out=bounce_in[lo:lo + half_m].rearrange(
                        "(p f) -> p f", f=cfg.B),
                    in_=masif_asm[mi][:])
            nc.gpsimd.collective_compute(
                "AllReduce", OP.add,
                replica_groups=[list(range(N_CORES))],
                ins=[bounce_in[:].opt()], outs=[bounce_out[:].opt()])

            pooled_gm = smp.tile([cfg.B, 2 * cfg.F], f32, tag='poolgm')
            nc.sync.dma_start(
                out=pooled_gm[:],
                in_=bounce_out[0:pf_n].rearrange("(p f) -> p f", f=2 * cfg.F))
            masif_rb = smp.tile([P, cfg.B], f32, tag='masrb')
            nc.sync.dma_start(
                out=masif_rb[:],
                in_=bounce_out[pf_n:ASM].rearrange("(p f) -> p f", f=cfg.B))

            # ---------------- head (replicated on all cores)
            x12 = {}
            for br in (1, 2):
                pfm = smp.tile([P, cfg.KC, cfg.B], f32, tag=f'pfm{br}')
                for k in range(cfg.KC):
                    tps = smps.tile([P, cfg.B], f32, space="PSUM", tag='sps')
                    nc.tensor.transpose(
                        out=tps[:],
                        in_=pooled_gm[:, (br - 1) * cfg.F + k * P:
                                      (br - 1) * cfg.F + (k + 1) * P],
                        identity=id32[:])
                    nc.scalar.activation(pfm[:, k, :], tps[:], AF.Identity)
                xps = smps.tile([P, cfg.B], f32, space="PSUM", tag='spsacc')
                for k in range(cfg.KC):
                    nc.tensor.matmul(xps[:], lhsT=wpf_sb[br][:, k, :],
                                     rhs=pfm[:, k, :],
                                     start=(k == 0), stop=(k == cfg.KC - 1))
                xs = smp.tile([P, cfg.B], f32, tag=f'x{br}')
                nc.scalar.activation(xs[:], xps[:], AF.Lrelu,
                                     bias=bpf_sb[br][:, 0:1], alpha=0.01)
                x12[br] = xs

            xc1 = {}
            for mh in range(2):
                cps = smps.tile([P, cfg.B], f32, space="PSUM", tag='spsacc')
                for k2 in range(2):
                    nc.tensor.matmul(
                        cps[:], lhsT=wfc1_sb[:, k2, mh * P:(mh + 1) * P],
                        rhs=x12[k2 + 1][:], start=(k2 == 0), stop=(k2 == 1))
                xcs = smp.tile([P, cfg.B], f32, tag=f'xc{mh}')
                nc.scalar.activation(xcs[:], cps[:], AF.Lrelu,
                                     bias=bfc1_sb[:, mh, 0:1], alpha=0.01)
                xc1[mh] = xcs
            c2ps = smps.tile([64, cfg.B], f32, space="PSUM", tag='spsacc')
            for k2 in range(2):
                nc.tensor.matmul(c2ps[:], lhsT=wfc2_sb[:, k2, :],
                                 rhs=xc1[k2][:], start=(k2 == 0), stop=(k2 == 1))
            xc = smp.tile([64, cfg.B], f32, tag='xcf')
            nc.scalar.activation(xc[:], c2ps[:], AF.Lrelu,
                                 bias=bfc2_sb[:, 0:1], alpha=0.01)

            ops = smps.tile([1, cfg.B], f32, space="PSUM", tag='spsacc')
            nc.tensor.matmul(ops[:], lhsT=wout_sb[0:64, 0, :], rhs=xc[:],
                             start=True, stop=False)
            nc.tensor.matmul(ops[:], lhsT=wout_sb[:, 1, :], rhs=masif_rb[:],
                             start=False, stop=True)
            res = smp.tile([1, cfg.B], f32, tag='res')
            nc.scalar.activation(res[:], ops[:], AF.Sigmoid,
                                 bias=bout_sb[:, 0:1])
            nc.sync.dma_start(out=out_t[:, :], in_=res[:])

    nc.compile()
    return nc


# ---------------------------------------------------------------- entry
_CACHE = {}


def _run(inputs, cfg, mm_dt=None, gs_dt=None, trace=False, tmpdir=None):
    import ml_dtypes
    from concourse import bass_utils
    mm_dt = mm_dt or ml_dtypes.bfloat16
    gs_dt = gs_dt or ml_dtypes.bfloat16
    meta, in_maps = _preprocess(inputs, cfg, mm_dt, gs_dt)
    key = (cfg.N, cfg.F, meta['cpad1'], meta['cpad2'],
           tuple(x[0] for x in meta['sched1']),
           tuple(x[0] for x in meta['sched2']),
           np.dtype(mm_dt).name, np.dtype(gs_dt).name)
    if key not in _CACHE:
        _CACHE.clear()
        _CACHE[key] = _build(cfg, meta, mm_dt, gs_dt)
    nc = _CACHE[key]
    res = bass_utils.run_bass_kernel_spmd(
        nc, in_maps, core_ids=list(range(N_CORES)), trace=trace, tmpdir=tmpdir)
    out = np.asarray(res.results[0]['out'], np.float32).reshape(cfg.B, 1)
    return out, res


def kernel(**inputs) -> np.ndarray:
    cfg = _Cfg()
    out, _ = _run(inputs, cfg)
    return out



# revision 3
# speedup vs baseline: 1.6585x; 1.6585x over previous
"""Trainium2 Bass kernel for nn_GCNN_87668872446200 (v2: fp8 + DoubleRow).

Two GCNConv+pool protein branches + two masif conv branches + dense head,
distributed over 8 NeuronCores as 2 feature-slices x 4 dest-node quarters.

Per core (all data in fp8 on the heavy paths):
  - xw = x @ W[:, fslice] via fp8 DoubleRow matmuls (2 k-tiles per pass),
    written to HBM as fp8 [NPAD, 512] (512B gather rows)
  - dma_gather pulls 512B source rows for this core's edge quarter
  - scatter-add as fp8 DoubleRow PE matmuls: S[256 edges, 128 dests]
    (host-built, norm-scaled) x gathered[256, 512] accumulated in PSUM
  - h = lrelu(psum + bias) [128, 512] bf16; transposed mean-pool via PE
    (pooledT [512, 32]) folding 1/cnt
  - x_pre = W_pf^T @ pooledT partial [128, 32] per branch (pre-activation,
    linear -> summable across cores)
  - masif branch: 4 graphs/core
  - ONE tiny AllReduce [128, 96] f32 (48KB: x_pre1|x_pre2|masif) at the end
  - transposed dense head (biases become per-partition) -> sigmoid -> [1, 32]

All consts ride in one packed uint8 arena (single DMA); idx+smat ride in a
combined per-call stream (one DMA per gather call). All 8 cores run ONE
identical program; per-core variation is in input data.
"""
import numpy as np

# ---------------------------------------------------------------- constants
N_CORES = 8
N_FSLICE = 2      # feature slices
N_DPART = 4       # destination-node partitions
P = 128
BLK = 128         # dest nodes per scatter block (S width)
PAIR_E = 256      # edges per DoubleRow matmul (2 chunks of 128)
GRP = 8           # chunks per dma_gather call (1024 idxs per call)
PAIRS_PER_CALL = GRP // 2

# problem sizes (hardcoded per spec)
N_NODES, N_EDGES, F_DIM, B_GRAPHS, L_MAS, C_MAS = 10000, 80000, 1024, 32, 800, 16


def _fp8():
    import ml_dtypes
    return ml_dtypes.float8_e4m3fn


def _bf16():
    import ml_dtypes
    return ml_dtypes.bfloat16


class _Cfg:
    def __init__(self, n=N_NODES, e=N_EDGES, f=F_DIM, b=B_GRAPHS,
                 l=L_MAS, c=C_MAS):
        assert f % 512 == 0 and b == 32 and l % 80 == 0 and c % 2 == 0
        self.N, self.E, self.F, self.B, self.L, self.C = n, e, f, b, l, c
        self.NPAD = ((n + 511) // 512) * 512
        while (self.NPAD // N_DPART) % BLK or (self.NPAD % 512):
            self.NPAD += 512
        self.HALF = self.NPAD // N_DPART       # nodes per dest partition
        self.NBLK = self.HALF // BLK           # blocks per dest partition
        self.FSL = f // N_FSLICE               # features per core slice
        self.KC = f // P                       # k-chunks of contraction
        self.KP = self.KC // 2                 # k-pairs (DoubleRow)
        self.NT = self.NPAD // 512             # node tiles
        self.GPB = b // N_CORES                # graphs per core for masif
        self.LW = l // 80                      # avg-pool window (10)
        self.LB = 8                            # l-blocks for masif layout
        self.LBS = l // self.LB                # l-block size (100)
        assert self.LBS % self.LW == 0
        self.WPB = self.LBS // self.LW         # windows per l-block (10)


# ------------------------------------------------------------- arena layout
# (name, rows, dtype-key, shape) -- shared by host packer and kernel views
def _arena_layout(cfg):
    return [
        ('wg1', 128, 'fp8', (cfg.KC, cfg.FSL)),
        ('wg2', 128, 'fp8', (cfg.KC, cfg.FSL)),
        ('bg1', 128, 'f32', (cfg.FSL,)),
        ('bg2', 128, 'f32', (cfg.FSL,)),
        ('mp1', 128, 'bf16', (cfg.NBLK, cfg.B)),
        ('mp2', 128, 'bf16', (cfg.NBLK, cfg.B)),
        ('wpf1', 128, 'f32', (4, 128)),
        ('wpf2', 128, 'f32', (4, 128)),
        ('bpf1', 128, 'f32', (1,)),
        ('bpf2', 128, 'f32', (1,)),
        ('wfc1', 128, 'f32', (2, 256)),
        ('bfc1', 128, 'f32', (2,)),
        ('wfc2', 128, 'f32', (2, 64)),
        ('bfc2', 64, 'f32', (1,)),
        ('wouta', 64, 'f32', (1,)),
        ('woutb', 128, 'f32', (1,)),
        ('bout', 1, 'f32', (1,)),
        ('wm1', 10, 'f32', (8, 64)),
        ('wm2', 10, 'f32', (8, 64)),
        ('bm1', 64, 'f32', (1,)),
        ('bm2', 64, 'f32', (1,)),
        ('scale_s1', 32, 'f32', (1,)),
        ('bias_s1', 32, 'f32', (1,)),
        ('scale_f1', 32, 'f32', (1,)),
        ('bias_f1', 32, 'f32', (1,)),
        ('scale_s2', 32, 'f32', (1,)),
        ('bias_s2', 32, 'f32', (1,)),
        ('scale_f2', 32, 'f32', (1,)),
        ('bias_f2', 32, 'f32', (1,)),
        ('gmask', 64, 'f32', (cfg.B,)),
    ]


def _dt_size(key):
    return {'f32': 4, 'bf16': 2, 'fp8': 1}[key]


def _arena_offsets(cfg):
    off, out = 0, {}
    for name, rows, key, shape in _arena_layout(cfg):
        nb = int(np.prod(shape)) * _dt_size(key)
        out[name] = (off, rows, key, shape, nb)
        off += (nb + 63) // 64 * 64
    return out, off


# ---------------------------------------------------------------- host prep
def _edge_plan(cfg, edge_index):
    """Per-quarter scatter plans, 256-edge pairs, shared pair schedule."""
    row = np.asarray(edge_index[0]).astype(np.int64)
    col = np.asarray(edge_index[1]).astype(np.int64)
    loops = np.arange(cfg.N, dtype=np.int64)
    rows = np.concatenate([row, loops])
    cols = np.concatenate([col, loops])
    deg = np.bincount(cols, minlength=cfg.N).astype(np.float64)
    dinv = 1.0 / np.sqrt(deg)
    norm = (dinv[rows] * dinv[cols]).astype(np.float32)

    per_hf = []
    pairs = np.zeros((N_DPART, cfg.NBLK), np.int64)
    for hf in range(N_DPART):
        lo = hf * cfg.HALF
        sel = (cols >= lo) & (cols < lo + cfg.HALF)
        r, c, w = rows[sel], cols[sel] - lo, norm[sel]
        order = np.argsort(c, kind='stable')
        r, c, w = r[order], c[order], w[order]
        blk = c // BLK
        starts = np.searchsorted(blk, np.arange(cfg.NBLK), 'left')
        ends = np.searchsorted(blk, np.arange(cfg.NBLK), 'right')
        pairs[hf] = np.maximum((ends - starts + PAIR_E - 1) // PAIR_E, 1)
        per_hf.append((r, c, w, starts, blk))
    kp = pairs.max(0)
    kp[-1] += (-int(kp.sum())) % PAIRS_PER_CALL
    npairs = int(kp.sum())
    nchunk = 2 * npairs

    sched = []
    for j in range(cfg.NBLK):
        for t in range(kp[j]):
            sched.append((j, t == 0, t == kp[j] - 1))
    assert len(sched) == npairs

    base = np.zeros(cfg.NBLK, np.int64)
    base[1:] = np.cumsum(2 * kp)[:-1]
    fp8 = _fp8()
    srcs_all, smat_all = [], []
    for hf in range(N_DPART):
        r, c, w, starts, blk = per_hf[hf]
        srcs = np.zeros((nchunk, P), np.int16)
        smat = np.zeros((nchunk, P, BLK), fp8)
        o = np.arange(len(r)) - starts[blk]
        ch = base[blk] + o // P
        sl = o % P
        srcs[ch, sl] = r.astype(np.int16)
        smat[ch, sl, c - blk * BLK] = w.astype(fp8)
        srcs_all.append(srcs)
        smat_all.append(smat)
    return sched, npairs, srcs_all, smat_all


def _wrap_idxs(srcs):
    """[C, 128] int16 -> wrapped [128, C*8] (idx j at [j%16 + 16*rep, j//16])."""
    flat = srcs.reshape(-1)
    w = flat.reshape(-1, 16).T                # [16, C*8]
    return np.ascontiguousarray(np.tile(w, (8, 1)).astype(np.int16))


def _build_scs(srcs, smat):
    """Combine wrapped idxs + grouped smat into one [calls, 128, 1152] u8."""
    nchunk = srcs.shape[0]
    calls = nchunk // GRP
    idxw = _wrap_idxs(srcs)                   # [128, nchunk*8] int16
    scs = np.zeros((calls, P, 128 + GRP * BLK), np.uint8)
    idxu = idxw.view(np.uint8).reshape(P, calls, 128).transpose(1, 0, 2)
    scs[:, :, 0:128] = idxu
    smu = smat.view(np.uint8).reshape(calls, GRP, P, BLK)
    scs[:, :, 128:] = smu.transpose(0, 2, 1, 3).reshape(calls, P, GRP * BLK)
    return scs


def _mpool(cfg, batch, hf):
    """[128, NBLK, B] bf16 folding 1/cnt, zero rows for pad nodes."""
    batch = np.asarray(batch).astype(np.int64)
    cnt = np.bincount(batch, minlength=cfg.B).astype(np.float64)
    cinv = (1.0 / np.maximum(cnt, 1.0)).astype(np.float32)
    m = np.zeros((P, cfg.NBLK, cfg.B), np.float32)
    lo = hf * cfg.HALF
    hi = min(lo + cfg.HALF, cfg.N)
    if hi > lo:
        nodes = np.arange(lo, hi)
        rel = nodes - lo
        m[rel % BLK, rel // BLK, batch[nodes]] = cinv[batch[nodes]]
    return m.astype(_bf16())


def _xtile(cfg, x):
    """[N, F] f32 -> [NT, 128, KC, 512] fp8 (partition=feat-in-chunk)."""
    fp8 = _fp8()
    xp = np.zeros((cfg.NPAD, cfg.F), fp8)
    xp[:cfg.N] = np.asarray(x, np.float32).astype(fp8)
    t = xp.reshape(cfg.NT, 512, cfg.KC, P).transpose(0, 3, 2, 1)
    return np.ascontiguousarray(t)


def _pack_arena(cfg, arrays):
    offs, total = _arena_offsets(cfg)
    ab = (total + 63) // 64 * 64
    arena = np.zeros((P, ab), np.uint8)
    for name, (off, rows, key, shape, nb) in offs.items():
        a = arrays[name]
        assert a.shape == (rows,) + tuple(shape), (name, a.shape, rows, shape)
        npdt = {'f32': np.float32, 'bf16': _bf16(), 'fp8': _fp8()}[key]
        flat = np.ascontiguousarray(a.astype(npdt)).view(np.uint8).reshape(rows, nb)
        arena[:rows, off:off + nb] = flat
    return arena


def _preprocess(inputs, cfg):
    fp8, bf16 = _fp8(), _bf16()
    meta = {}
    shared_plans = {}
    xtiles = {}
    for br in (1, 2):
        xtiles[br] = _xtile(cfg, inputs[f'pro{br}_x'])
        sched, npairs, srcs, smat = _edge_plan(cfg, inputs[f'pro{br}_edge_index'])
        meta[f'sched{br}'] = sched
        meta[f'npairs{br}'] = npairs
        shared_plans[br] = (srcs, smat)

    W_out = np.asarray(inputs['W_out'], np.float32)

    def f32(v):
        return np.asarray(v, np.float32)

    in_maps = []
    for core in range(N_CORES):
        fs, hf = core % N_FSLICE, core // N_FSLICE
        f_lo = fs * cfg.FSL
        ar = {}
        for br in (1, 2):
            Wg = f32(inputs[f'W_g{br}'])
            ar[f'wg{br}'] = np.ascontiguousarray(
                Wg.reshape(cfg.KC, P, cfg.F)[:, :, f_lo:f_lo + cfg.FSL]
                .transpose(1, 0, 2)).astype(fp8)
            bg = f32(inputs[f'b_g{br}'])[f_lo:f_lo + cfg.FSL]
            ar[f'bg{br}'] = np.tile(bg[None, :], (P, 1))
            ar[f'mp{br}'] = _mpool(cfg, inputs[f'pro{br}_batch'], hf)
            Wpf = f32(inputs[f'W_pf{br}'])[f_lo:f_lo + cfg.FSL]
            ar[f'wpf{br}'] = np.ascontiguousarray(
                Wpf.reshape(4, P, P).transpose(1, 0, 2))
            ar[f'bpf{br}'] = f32(inputs[f'b_pf{br}']).reshape(P, 1)
            ar[f'wm{br}'] = np.ascontiguousarray(
                (f32(inputs[f'W_m{br}']) / (2.0 * cfg.LW))
                .reshape(8, 10, 64).transpose(1, 0, 2))
            ar[f'bm{br}'] = f32(inputs[f'b_m{br}']).reshape(64, 1)
            for sf, pre in (('s', 'cs'), ('f', 'cf')):
                w = float(np.asarray(inputs[f'{pre}{br}_w'])[0])
                b = float(np.asarray(inputs[f'{pre}{br}_b'])[0])
                ar[f'scale_{sf}{br}'] = np.full((32, 1), w / cfg.C, np.float32)
                ar[f'bias_{sf}{br}'] = np.full((32, 1), b, np.float32)
        ar['wfc1'] = np.ascontiguousarray(
            f32(inputs['W_fc1']).reshape(2, P, 256).transpose(1, 0, 2))
        ar['bfc1'] = np.ascontiguousarray(f32(inputs['b_fc1']).reshape(2, P).T)
        ar['wfc2'] = np.ascontiguousarray(
            f32(inputs['W_fc2']).reshape(2, P, 64).transpose(1, 0, 2))
        ar['bfc2'] = f32(inputs['b_fc2']).reshape(64, 1)
        ar['wouta'] = W_out[0:64].reshape(64, 1)
        ar['woutb'] = W_out[64:192].reshape(P, 1)
        ar['bout'] = f32(inputs['b_out']).reshape(1, 1)
        gm = np.zeros((64, cfg.B), np.float32)
        gm[:, core * cfg.GPB:(core + 1) * cfg.GPB] = 1.0
        ar['gmask'] = gm

        arena = _pack_arena(cfg, ar)

        # masif arena2: 4 tensors x [32, 16, 100] f32 = 4 x 6400B per row
        a2 = np.zeros((32, 4 * 6400), np.uint8)
        for ti, name in enumerate(['mas1_straight', 'mas1_flipped',
                                   'mas2_straight', 'mas2_flipped']):
            a = f32(inputs[name])[core * cfg.GPB:(core + 1) * cfg.GPB]
            blk = a.reshape(cfg.GPB, cfg.C, cfg.LB, cfg.LBS) \
                   .transpose(2, 0, 1, 3).reshape(32, cfg.C * cfg.LBS)
            a2[:, ti * 6400:(ti + 1) * 6400] = \
                np.ascontiguousarray(blk).view(np.uint8)

        m = {'arena': arena, 'arena2': a2}
        for br in (1, 2):
            m[f'xT{br}'] = xtiles[br]
            srcs, smat = shared_plans[br]
            m[f'scs{br}'] = _build_scs(srcs[hf], smat[hf])
        in_maps.append(m)
    return meta, in_maps


# ---------------------------------------------------------------- program
def _build(cfg, meta):
    import concourse.bass as bass
    import concourse.bacc as bacc
    import concourse.mybir as mybir
    import concourse.tile as tile
    from concourse.masks import make_identity

    dt = mybir.dt
    fp8 = dt.float8e4
    bf16 = dt.bfloat16
    f32 = dt.float32
    u8 = dt.uint8
    AF = mybir.ActivationFunctionType
    OP = mybir.AluOpType
    DR = mybir.MatmulPerfMode.DoubleRow

    nc = bacc.Bacc("TRN2", target_bir_lowering=False, debug=False,
                   enable_asserts=False, num_devices=N_CORES,
                   num_swdge_queues=2)

    offs, total = _arena_offsets(cfg)
    AB = (total + 63) // 64 * 64

    arena_d = nc.dram_tensor('arena', [P, AB], u8, kind="ExternalInput")
    arena2_d = nc.dram_tensor('arena2', [32, 4 * 6400], u8, kind="ExternalInput")
    xT = {br: nc.dram_tensor(f'xT{br}', [cfg.NT, P, cfg.KC, 512], fp8,
                             kind="ExternalInput") for br in (1, 2)}
    scs_d = {br: nc.dram_tensor(
        f'scs{br}', [meta[f'npairs{br}'] // PAIRS_PER_CALL, P, 128 + GRP * BLK],
        u8, kind="ExternalInput") for br in (1, 2)}
    out_t = nc.dram_tensor('out', [1, cfg.B], f32, kind="ExternalOutput")

    with tile.TileContext(nc) as tc:
        with tc.tile_pool(name="const", bufs=1) as cst, \
             tc.tile_pool(name="xt", bufs=2) as xtp, \
             tc.tile_pool(name="xwps", bufs=2, space="PSUM") as xwps, \
             tc.tile_pool(name="xwsb", bufs=3) as xwsb, \
             tc.tile_pool(name="scs", bufs=4) as scsp, \
             tc.tile_pool(name="gat", bufs=4) as gatp, \
             tc.tile_pool(name="blkps", bufs=2, space="PSUM") as blkps, \
             tc.tile_pool(name="hb", bufs=3) as hp, \
             tc.tile_pool(name="poolps", bufs=2, space="PSUM") as poolp, \
             tc.tile_pool(name="small", bufs=2) as smp, \
             tc.tile_pool(name="smallps", bufs=1, space="PSUM") as smps, \
             tc.tile_pool(name="dram", bufs=2, space="DRAM") as drp:

            # ---------------- constant arena (one DMA each)
            arena_t = cst.tile([P, AB], u8, tag='arena')
            nc.sync.dma_start(out=arena_t[:], in_=arena_d.ap())
            arena2_t = cst.tile([32, 4 * 6400], u8, tag='arena2')
            nc.sync.dma_start(out=arena2_t[:], in_=arena2_d.ap())

            def av(name, dtype):
                off, rows, key, shape, nb = offs[name]
                v = arena_t[0:rows, off:off + nb].bitcast(dtype)
                if len(shape) == 2:
                    v = v.rearrange("p (a b) -> p a b", a=shape[0])
                return v

            wg_v = {br: av(f'wg{br}', fp8) for br in (1, 2)}
            bg_v = {br: av(f'bg{br}', f32) for br in (1, 2)}
            mp_v = {br: av(f'mp{br}', bf16) for br in (1, 2)}
            wpf_v = {br: av(f'wpf{br}', f32) for br in (1, 2)}
            bpf_v = {br: av(f'bpf{br}', f32) for br in (1, 2)}
            wfc1_v = av('wfc1', f32)
            bfc1_v = av('bfc1', f32)
            wfc2_v = av('wfc2', f32)
            bfc2_v = av('bfc2', f32)
            wouta_v = av('wouta', f32)
            woutb_v = av('woutb', f32)
            bout_v = av('bout', f32)
            wm_v = {br: av(f'wm{br}', f32) for br in (1, 2)}
            bm_v = {br: av(f'bm{br}', f32) for br in (1, 2)}
            msc_v = {(br, sf, kind): av(f'{kind}_{sf}{br}', f32)
                     for br in (1, 2) for sf in 'sf'
                     for kind in ('scale', 'bias')}
            gmask_v = av('gmask', f32)

            id32 = cst.tile([32, 32], f32, tag='id32')
            make_identity(nc, id32[:])

            # ---------------- masif (both branches) -> two [64, B] f32 tiles
            masif_asm = {}
            for mi in (1, 2):
                frag = None
                for si, sf in enumerate('sf'):
                    toff = ((mi - 1) * 2 + si) * 6400
                    mv = arena2_t[:, toff:toff + 6400].bitcast(f32) \
                        .rearrange("p (c l) -> p c l", c=cfg.C)
                    red = smp.tile([32, cfg.LBS], f32, tag='masred')
                    nc.vector.tensor_reduce(
                        out=red[:], in_=mv.transpose([0, 2, 1]),
                        axis=mybir.AxisListType.X, op=OP.add)
                    act = smp.tile([32, cfg.LBS], f32, tag='masact')
                    nc.scalar.activation(
                        act[:], red[:], AF.Relu,
                        bias=msc_v[(mi, sf, 'bias')][:, 0:1],
                        scale=msc_v[(mi, sf, 'scale')][:, 0:1])
                    ws = smp.tile([32, cfg.WPB], f32, tag='masws')
                    nc.vector.tensor_reduce(
                        out=ws[:],
                        in_=act[:].rearrange("p (w l) -> p w l", l=cfg.LW),
                        axis=mybir.AxisListType.X, op=OP.add)
                    if frag is None:
                        frag = ws
                    else:
                        frag2 = smp.tile([32, cfg.WPB], f32, tag='masfrag')
                        nc.vector.tensor_add(out=frag2[:], in0=frag[:],
                                             in1=ws[:])
                        frag = frag2
                ps_t = smps.tile([cfg.WPB, 32], f32, space="PSUM", tag='spsA')
                nc.tensor.transpose(out=ps_t[:], in_=frag[:], identity=id32[:])
                fragT = smp.tile([cfg.WPB, 32], f32, tag='masfragT')
                nc.scalar.activation(fragT[:], ps_t[:], AF.Identity)
                fragTc = fragT[:].rearrange("k (lb g) -> k lb g", g=cfg.GPB)
                m_ps = smps.tile([64, cfg.GPB], f32, space="PSUM", tag='spsB')
                for lb in range(cfg.LB):
                    nc.tensor.matmul(
                        m_ps[:], lhsT=wm_v[mi][:, lb, :], rhs=fragTc[:, lb, :],
                        start=(lb == 0), stop=(lb == cfg.LB - 1))
                m_fm = smp.tile([64, cfg.GPB], f32, tag='masfm')
                nc.scalar.activation(m_fm[:], m_ps[:], AF.Identity,
                                     bias=bm_v[mi][:, 0:1])
                masm = cst.tile([64, cfg.B], f32, tag=f'masasm{mi}')
                nc.vector.tensor_tensor(
                    out=masm[:].rearrange("p (s g) -> p s g", g=cfg.GPB),
                    in0=m_fm[:, None, :].to_broadcast(
                        [64, N_CORES, cfg.GPB]),
                    in1=gmask_v.rearrange("p (s g) -> p s g", g=cfg.GPB),
                    op=OP.mult)
                masif_asm[mi] = masm

            # ---------------- allreduce bounce buffers
            ASM = P * 96
            bounce_in = drp.tile([ASM], f32, tag='ccin')
            bounce_out = drp.tile([ASM], f32, tag='ccout')
            bi_v = bounce_in[:].rearrange("(p f) -> p f", f=96)
            nc.sync.dma_start(out=bi_v[0:64, 64:96], in_=masif_asm[1][:])
            nc.sync.dma_start(out=bi_v[64:128, 64:96], in_=masif_asm[2][:])

            # ---------------- GCN branches
            for br in (1, 2):
                npairs = meta[f'npairs{br}']
                sched = meta[f'sched{br}']
                n_call = npairs // PAIRS_PER_CALL
                xw_dram = drp.tile([cfg.NPAD, cfg.FSL], fp8, tag='xwdram')

                # xw = x @ W slice (fp8 DoubleRow, x stationary)
                for nt in range(cfg.NT):
                    xt_t = xtp.tile([P, cfg.KC, 512], fp8, tag='xt')
                    nc.sync.dma_start(out=xt_t[:], in_=xT[br].ap()[nt])
                    xw_t = xwsb.tile([P, 4, cfg.FSL], fp8, tag='xwsb')
                    for sub in range(4):
                        ps = xwps.tile([P, cfg.FSL], f32, space="PSUM",
                                       tag='xwps')
                        for kp in range(cfg.KP):
                            nc.tensor.matmul(
                                ps[:],
                                lhsT=xt_t[:, 2 * kp:2 * kp + 2,
                                          sub * P:(sub + 1) * P],
                                rhs=wg_v[br][:, 2 * kp:2 * kp + 2, :],
                                start=(kp == 0), stop=(kp == cfg.KP - 1),
                                perf_mode=DR)
                        if sub % 2 == 0:
                            nc.vector.tensor_scalar_mul(
                                xw_t[:, sub, :], ps[:], 1.0)
                        else:
                            nc.scalar.activation(
                                xw_t[:, sub, :], ps[:], AF.Identity)
                    nc.sync.dma_start(
                        out=xw_dram[nt * 512:(nt + 1) * 512, :].rearrange(
                            "(s p) f -> p s f", p=P),
                        in_=xw_t[:])

                # scatter + transposed pool
                poolps = poolp.tile([P, 4, cfg.B], f32, space="PSUM",
                                    tag='poolps')
                pi = 0
                blk_ps = None
                for g in range(n_call):
                    scs_t = scsp.tile([P, 128 + GRP * BLK], u8, tag='scs')
                    nc.sync.dma_start(out=scs_t[:], in_=scs_d[br].ap()[g])
                    idx_v = scs_t[:, 0:128].bitcast(dt.int16)
                    s_v = scs_t[:, 128:128 + GRP * BLK].bitcast(fp8) \
                        .rearrange("p (c d) -> p c d", c=GRP)
                    gat_t = gatp.tile([P, GRP, cfg.FSL], fp8, tag='gat')
                    nc.gpsimd.dma_gather(
                        out_ap=gat_t[:], in_ap=xw_dram[:, :], idxs_ap=idx_v,
                        num_idxs=GRP * P, num_idxs_reg=GRP * P,
                        elem_size=cfg.FSL, queue_num=g % 2)
                    for i in range(PAIRS_PER_CALL):
                        j, st, sp = sched[pi]
                        if st:
                            blk_ps = blkps.tile([P, cfg.FSL], f32,
                                                space="PSUM", tag='blkps')
                        nc.tensor.matmul(
                            blk_ps[:],
                            lhsT=s_v[:, 2 * i:2 * i + 2, :],
                            rhs=gat_t[:, 2 * i:2 * i + 2, :],
                            start=st, stop=sp, perf_mode=DR)
                        if sp:
                            h_t = hp.tile([P, cfg.FSL], bf16, tag='h')
                            nc.vector.tensor_add(out=h_t[:], in0=blk_ps[:],
                                                 in1=bg_v[br])
                            nc.scalar.activation(h_t[:], h_t[:], AF.Lrelu,
                                                 alpha=0.01)
                            for c in range(4):
                                nc.tensor.matmul(
                                    poolps[:, c, :],
                                    lhsT=h_t[:, c * P:(c + 1) * P],
                                    rhs=mp_v[br][:, j, :],
                                    start=(j == 0), stop=(j == cfg.NBLK - 1),
                                    skip_group_check=True)
                        pi += 1

                # x_pre partial: [128, B] = W_pf^T @ pooledT
                pooled_sb = smp.tile([P, 4, cfg.B], f32, tag='pooled')
                nc.vector.tensor_scalar_mul(pooled_sb[:], poolps[:], 1.0)
                xpre_ps = smps.tile([P, cfg.B], f32, space="PSUM", tag='spsA')
                for c in range(4):
                    nc.tensor.matmul(xpre_ps[:], lhsT=wpf_v[br][:, c, :],
                                     rhs=pooled_sb[:, c, :],
                                     start=(c == 0), stop=(c == 3))
                xpre_sb = smp.tile([P, cfg.B], f32, tag='xpresb')
                nc.vector.tensor_scalar_mul(xpre_sb[:], xpre_ps[:], 1.0)
                nc.sync.dma_start(
                    out=bi_v[:, (br - 1) * cfg.B:br * cfg.B], in_=xpre_sb[:])

            # ---------------- tiny allreduce (48KB)
            nc.gpsimd.collective_compute(
                "AllReduce", OP.add,
                replica_groups=[list(range(N_CORES))],
                ins=[bounce_in[:].opt()], outs=[bounce_out[:].opt()])

            xr = smp.tile([P, 96], f32, tag='xr')
            nc.sync.dma_start(
                out=xr[:],
                in_=bounce_out[:].rearrange("(p f) -> p f", f=96))

            # ---------------- head (transposed; replicated on all cores)
            x12 = smp.tile([P, 2 * cfg.B], f32, tag='x12')
            for br in (1, 2):
                nc.scalar.activation(
                    x12[:, (br - 1) * cfg.B:br * cfg.B],
                    xr[:, (br - 1) * cfg.B:br * cfg.B],
                    AF.Lrelu, bias=bpf_v[br][:, 0:1], alpha=0.01)
            xc1 = smp.tile([P, 2 * cfg.B], f32, tag='xc1')
            for mh in range(2):
                cps = smps.tile([P, cfg.B], f32, space="PSUM", tag='spsA')
                for k2 in range(2):
                    nc.tensor.matmul(
                        cps[:], lhsT=wfc1_v[:, k2, mh * P:(mh + 1) * P],
                        rhs=x12[:, k2 * cfg.B:(k2 + 1) * cfg.B],
                        start=(k2 == 0), stop=(k2 == 1))
                nc.scalar.activation(
                    xc1[:, mh * cfg.B:(mh + 1) * cfg.B], cps[:], AF.Lrelu,
                    bias=bfc1_v[:, mh:mh + 1], alpha=0.01)
            c2ps = smps.tile([64, cfg.B], f32, space="PSUM", tag='spsB')
            for k2 in range(2):
                nc.tensor.matmul(c2ps[:], lhsT=wfc2_v[:, k2, :],
                                 rhs=xc1[:, k2 * cfg.B:(k2 + 1) * cfg.B],
                                 start=(k2 == 0), stop=(k2 == 1))
            xc2 = smp.tile([64, cfg.B], f32, tag='xc2')
            nc.scalar.activation(xc2[:], c2ps[:], AF.Lrelu,
                                 bias=bfc2_v[:, 0:1], alpha=0.01)

            zps = smps.tile([1, cfg.B], f32, space="PSUM", tag='spsA')
            nc.tensor.matmul(zps[:], lhsT=wouta_v[0:64, :], rhs=xc2[:],
                             start=True, stop=False)
            nc.tensor.matmul(zps[:], lhsT=woutb_v[:, :], rhs=xr[:, 64:96],
                             start=False, stop=True)
            res = smp.tile([1, cfg.B], f32, tag='res')
            nc.scalar.activation(res[:], zps[:], AF.Sigmoid,
                                 bias=bout_v[:, 0:1])
            nc.sync.dma_start(out=out_t[:, :], in_=res[:])

    nc.compile()
    return nc


# ---------------------------------------------------------------- entry
_CACHE = {}


def _run(inputs, cfg, trace=False, tmpdir=None):
    from concourse import bass_utils
    meta, in_maps = _preprocess(inputs, cfg)
    key = (cfg.N, cfg.F, meta['npairs1'], meta['npairs2'],
           tuple(x[0] for x in meta['sched1']),
           tuple(x[0] for x in meta['sched2']))
    if key not in _CACHE:
        _CACHE.clear()
        _CACHE[key] = _build(cfg, meta)
    nc = _CACHE[key]
    res = bass_utils.run_bass_kernel_spmd(
        nc, in_maps, core_ids=list(range(N_CORES)), trace=trace, tmpdir=tmpdir)
    out = np.asarray(res.results[0]['out'], np.float32).reshape(cfg.B, 1)
    return out, res


def kernel(**inputs) -> np.ndarray:
    cfg = _Cfg()
    out, _ = _run(inputs, cfg)
    return out


# revision 4
# speedup vs baseline: 2.4666x; 1.4873x over previous
"""Trainium2 Bass kernel for nn_GCNN_87668872446200 (v2: fp8 + DoubleRow).

Two GCNConv+pool protein branches + two masif conv branches + dense head,
distributed over 8 NeuronCores as 2 feature-slices x 4 dest-node quarters.

Per core (all data in fp8 on the heavy paths):
  - xw = x @ W[:, fslice] via fp8 DoubleRow matmuls (2 k-tiles per pass),
    written to HBM as fp8 [NPAD, 512] (512B gather rows)
  - dma_gather pulls 512B source rows for this core's edge quarter
  - scatter-add as fp8 DoubleRow PE matmuls: S[256 edges, 128 dests]
    (host-built, norm-scaled) x gathered[256, 512] accumulated in PSUM
  - h = lrelu(psum + bias) [128, 512] bf16; transposed mean-pool via PE
    (pooledT [512, 32]) folding 1/cnt
  - x_pre = W_pf^T @ pooledT partial [128, 32] per branch (pre-activation,
    linear -> summable across cores)
  - masif branch: 4 graphs/core
  - ONE tiny AllReduce [128, 96] f32 (48KB: x_pre1|x_pre2|masif) at the end
  - transposed dense head (biases become per-partition) -> sigmoid -> [1, 32]

All consts ride in one packed uint8 arena (single DMA); idx+smat ride in a
combined per-call stream (one DMA per gather call). All 8 cores run ONE
identical program; per-core variation is in input data.
"""
import numpy as np

# ---------------------------------------------------------------- constants
N_CORES = 8
N_FSLICE = 2      # feature slices
N_DPART = 4       # destination-node partitions
P = 128
BLK = 128         # dest nodes per scatter block (S width)
PAIR_E = 256      # edges per DoubleRow matmul (2 chunks of 128)
GRP = 8           # chunks per dma_gather call (1024 idxs per call)
PAIRS_PER_CALL = GRP // 2

# problem sizes (hardcoded per spec)
N_NODES, N_EDGES, F_DIM, B_GRAPHS, L_MAS, C_MAS = 10000, 80000, 1024, 32, 800, 16


def _fp8():
    import ml_dtypes
    return ml_dtypes.float8_e4m3fn


def _bf16():
    import ml_dtypes
    return ml_dtypes.bfloat16


class _Cfg:
    def __init__(self, n=N_NODES, e=N_EDGES, f=F_DIM, b=B_GRAPHS,
                 l=L_MAS, c=C_MAS):
        assert f % 512 == 0 and b == 32 and l % 80 == 0 and c % 2 == 0
        self.N, self.E, self.F, self.B, self.L, self.C = n, e, f, b, l, c
        self.NPAD = ((n + 511) // 512) * 512
        while (self.NPAD // N_DPART) % BLK or (self.NPAD % 512):
            self.NPAD += 512
        self.HALF = self.NPAD // N_DPART       # nodes per dest partition
        self.NBLK = self.HALF // BLK           # blocks per dest partition
        self.FSL = f // N_FSLICE               # features per core slice
        self.KC = f // P                       # k-chunks of contraction
        self.KP = self.KC // 2                 # k-pairs (DoubleRow)
        self.NT = self.NPAD // 512             # node tiles
        self.GPB = b // N_CORES                # graphs per core for masif
        self.LW = l // 80                      # avg-pool window (10)
        self.LB = 8                            # l-blocks for masif layout
        self.LBS = l // self.LB                # l-block size (100)
        assert self.LBS % self.LW == 0
        self.WPB = self.LBS // self.LW         # windows per l-block (10)


# ------------------------------------------------------------- arena layout
# (name, rows, dtype-key, shape) -- shared by host packer and kernel views
def _arena_layout(cfg):
    return [
        ('wg1', 128, 'fp8', (cfg.KC, cfg.FSL)),
        ('wg2', 128, 'fp8', (cfg.KC, cfg.FSL)),
        ('bg1', 128, 'f32', (cfg.FSL,)),
        ('bg2', 128, 'f32', (cfg.FSL,)),
        ('mp1', 128, 'bf16', (cfg.NBLK, cfg.B)),
        ('mp2', 128, 'bf16', (cfg.NBLK, cfg.B)),
        ('wpf1', 128, 'f32', (4, 128)),
        ('wpf2', 128, 'f32', (4, 128)),
        ('wm1', 10, 'f32', (8, 64)),
        ('wm2', 10, 'f32', (8, 64)),
        ('bm1', 64, 'f32', (1,)),
        ('bm2', 64, 'f32', (1,)),
        ('scale_s1', 32, 'f32', (1,)),
        ('bias_s1', 32, 'f32', (1,)),
        ('scale_f1', 32, 'f32', (1,)),
        ('bias_f1', 32, 'f32', (1,)),
        ('scale_s2', 32, 'f32', (1,)),
        ('bias_s2', 32, 'f32', (1,)),
        ('scale_f2', 32, 'f32', (1,)),
        ('bias_f2', 32, 'f32', (1,)),
        ('gmask', 64, 'f32', (cfg.B,)),
    ]


def _dt_size(key):
    return {'f32': 4, 'bf16': 2, 'fp8': 1}[key]


def _arena_offsets(cfg):
    off, out = 0, {}
    for name, rows, key, shape in _arena_layout(cfg):
        nb = int(np.prod(shape)) * _dt_size(key)
        out[name] = (off, rows, key, shape, nb)
        off += (nb + 63) // 64 * 64
    return out, off


# ---------------------------------------------------------------- host prep
def _edge_plan(cfg, edge_index):
    """Per-quarter scatter plans, 256-edge pairs, shared pair schedule."""
    row = np.asarray(edge_index[0]).astype(np.int64)
    col = np.asarray(edge_index[1]).astype(np.int64)
    loops = np.arange(cfg.N, dtype=np.int64)
    rows = np.concatenate([row, loops])
    cols = np.concatenate([col, loops])
    deg = np.bincount(cols, minlength=cfg.N).astype(np.float64)
    dinv = 1.0 / np.sqrt(deg)
    norm = (dinv[rows] * dinv[cols]).astype(np.float32)

    per_hf = []
    pairs = np.zeros((N_DPART, cfg.NBLK), np.int64)
    for hf in range(N_DPART):
        lo = hf * cfg.HALF
        sel = (cols >= lo) & (cols < lo + cfg.HALF)
        r, c, w = rows[sel], cols[sel] - lo, norm[sel]
        order = np.argsort(c, kind='stable')
        r, c, w = r[order], c[order], w[order]
        blk = c // BLK
        starts = np.searchsorted(blk, np.arange(cfg.NBLK), 'left')
        ends = np.searchsorted(blk, np.arange(cfg.NBLK), 'right')
        pairs[hf] = np.maximum((ends - starts + PAIR_E - 1) // PAIR_E, 1)
        per_hf.append((r, c, w, starts, blk))
    kp = pairs.max(0)
    kp[-1] += (-int(kp.sum())) % PAIRS_PER_CALL
    npairs = int(kp.sum())
    nchunk = 2 * npairs

    sched = []
    for j in range(cfg.NBLK):
        for t in range(kp[j]):
            sched.append((j, t == 0, t == kp[j] - 1))
    assert len(sched) == npairs

    base = np.zeros(cfg.NBLK, np.int64)
    base[1:] = np.cumsum(2 * kp)[:-1]
    fp8 = _fp8()
    srcs_all, smat_all = [], []
    for hf in range(N_DPART):
        r, c, w, starts, blk = per_hf[hf]
        srcs = np.zeros((nchunk, P), np.int16)
        smat = np.zeros((nchunk, P, BLK), fp8)
        o = np.arange(len(r)) - starts[blk]
        ch = base[blk] + o // P
        sl = o % P
        srcs[ch, sl] = r.astype(np.int16)
        smat[ch, sl, c - blk * BLK] = w.astype(fp8)
        srcs_all.append(srcs)
        smat_all.append(smat)
    return sched, npairs, srcs_all, smat_all


def _wrap_idxs(srcs):
    """[C, 128] int16 -> wrapped [128, C*8] (idx j at [j%16 + 16*rep, j//16])."""
    flat = srcs.reshape(-1)
    w = flat.reshape(-1, 16).T                # [16, C*8]
    return np.ascontiguousarray(np.tile(w, (8, 1)).astype(np.int16))


def _build_scs(srcs, smat):
    """Combine wrapped idxs + grouped smat into one [calls, 128, 1152] u8."""
    nchunk = srcs.shape[0]
    calls = nchunk // GRP
    idxw = _wrap_idxs(srcs)                   # [128, nchunk*8] int16
    scs = np.zeros((calls, P, 128 + GRP * BLK), np.uint8)
    idxu = idxw.view(np.uint8).reshape(P, calls, 128).transpose(1, 0, 2)
    scs[:, :, 0:128] = idxu
    smu = smat.view(np.uint8).reshape(calls, GRP, P, BLK)
    scs[:, :, 128:] = smu.transpose(0, 2, 1, 3).reshape(calls, P, GRP * BLK)
    return scs


def _mpool(cfg, batch, hf):
    """[128, NBLK, B] bf16 folding 1/cnt, zero rows for pad nodes."""
    batch = np.asarray(batch).astype(np.int64)
    cnt = np.bincount(batch, minlength=cfg.B).astype(np.float64)
    cinv = (1.0 / np.maximum(cnt, 1.0)).astype(np.float32)
    m = np.zeros((P, cfg.NBLK, cfg.B), np.float32)
    lo = hf * cfg.HALF
    hi = min(lo + cfg.HALF, cfg.N)
    if hi > lo:
        nodes = np.arange(lo, hi)
        rel = nodes - lo
        m[rel % BLK, rel // BLK, batch[nodes]] = cinv[batch[nodes]]
    return m.astype(_bf16())


def _xtile(cfg, x):
    """[N, F] f32 -> [NT, 128, KC, 512] fp8 (partition=feat-in-chunk)."""
    fp8 = _fp8()
    xp = np.zeros((cfg.NPAD, cfg.F), fp8)
    xp[:cfg.N] = np.asarray(x, np.float32).astype(fp8)
    t = xp.reshape(cfg.NT, 512, cfg.KC, P).transpose(0, 3, 2, 1)
    return np.ascontiguousarray(t)


def _pack_arena(cfg, arrays):
    offs, total = _arena_offsets(cfg)
    ab = (total + 63) // 64 * 64
    arena = np.zeros((P, ab), np.uint8)
    for name, (off, rows, key, shape, nb) in offs.items():
        a = arrays[name]
        assert a.shape == (rows,) + tuple(shape), (name, a.shape, rows, shape)
        npdt = {'f32': np.float32, 'bf16': _bf16(), 'fp8': _fp8()}[key]
        flat = np.ascontiguousarray(a.astype(npdt)).view(np.uint8).reshape(rows, nb)
        arena[:rows, off:off + nb] = flat
    return arena


def _preprocess(inputs, cfg):
    fp8, bf16 = _fp8(), _bf16()
    meta = {}
    shared_plans = {}
    xtiles = {}
    for br in (1, 2):
        xtiles[br] = _xtile(cfg, inputs[f'pro{br}_x'])
        sched, npairs, srcs, smat = _edge_plan(cfg, inputs[f'pro{br}_edge_index'])
        meta[f'sched{br}'] = sched
        meta[f'npairs{br}'] = npairs
        shared_plans[br] = (srcs, smat)

    def f32(v):
        return np.asarray(v, np.float32)

    in_maps = []
    for core in range(N_CORES):
        fs, hf = core % N_FSLICE, core // N_FSLICE
        f_lo = fs * cfg.FSL
        ar = {}
        for br in (1, 2):
            Wg = f32(inputs[f'W_g{br}'])
            ar[f'wg{br}'] = np.ascontiguousarray(
                Wg.reshape(cfg.KC, P, cfg.F)[:, :, f_lo:f_lo + cfg.FSL]
                .transpose(1, 0, 2)).astype(fp8)
            bg = f32(inputs[f'b_g{br}'])[f_lo:f_lo + cfg.FSL]
            ar[f'bg{br}'] = np.tile(bg[None, :], (P, 1))
            ar[f'mp{br}'] = _mpool(cfg, inputs[f'pro{br}_batch'], hf)
            Wpf = f32(inputs[f'W_pf{br}'])[f_lo:f_lo + cfg.FSL]
            ar[f'wpf{br}'] = np.ascontiguousarray(
                Wpf.reshape(4, P, P).transpose(1, 0, 2))
            ar[f'wm{br}'] = np.ascontiguousarray(
                (f32(inputs[f'W_m{br}']) / (2.0 * cfg.LW))
                .reshape(8, 10, 64).transpose(1, 0, 2))
            ar[f'bm{br}'] = f32(inputs[f'b_m{br}']).reshape(64, 1)
            for sf, pre in (('s', 'cs'), ('f', 'cf')):
                w = float(np.asarray(inputs[f'{pre}{br}_w'])[0])
                b = float(np.asarray(inputs[f'{pre}{br}_b'])[0])
                ar[f'scale_{sf}{br}'] = np.full((32, 1), w / cfg.C, np.float32)
                ar[f'bias_{sf}{br}'] = np.full((32, 1), b, np.float32)
        gm = np.zeros((64, cfg.B), np.float32)
        gm[:, core * cfg.GPB:(core + 1) * cfg.GPB] = 1.0
        ar['gmask'] = gm

        arena = _pack_arena(cfg, ar)

        # masif arena2: 4 tensors x [32, 16, 100] f32 = 4 x 6400B per row
        a2 = np.zeros((32, 4 * 6400), np.uint8)
        for ti, name in enumerate(['mas1_straight', 'mas1_flipped',
                                   'mas2_straight', 'mas2_flipped']):
            a = f32(inputs[name])[core * cfg.GPB:(core + 1) * cfg.GPB]
            blk = a.reshape(cfg.GPB, cfg.C, cfg.LB, cfg.LBS) \
                   .transpose(2, 0, 1, 3).reshape(32, cfg.C * cfg.LBS)
            a2[:, ti * 6400:(ti + 1) * 6400] = \
                np.ascontiguousarray(blk).view(np.uint8)

        m = {'arena': arena, 'arena2': a2}
        for br in (1, 2):
            m[f'xT{br}'] = xtiles[br]
            srcs, smat = shared_plans[br]
            m[f'scs{br}'] = _build_scs(srcs[hf], smat[hf])
        in_maps.append(m)
    return meta, in_maps


# ---------------------------------------------------------------- program
def _build(cfg, meta):
    import concourse.bass as bass
    import concourse.bacc as bacc
    import concourse.mybir as mybir
    import concourse.tile as tile
    from concourse.masks import make_identity

    dt = mybir.dt
    fp8 = dt.float8e4
    bf16 = dt.bfloat16
    f32 = dt.float32
    u8 = dt.uint8
    AF = mybir.ActivationFunctionType
    OP = mybir.AluOpType
    DR = mybir.MatmulPerfMode.DoubleRow

    nc = bacc.Bacc("TRN2", target_bir_lowering=False, debug=False,
                   enable_asserts=False, num_devices=N_CORES,
                   num_swdge_queues=2)

    offs, total = _arena_offsets(cfg)
    AB = (total + 63) // 64 * 64

    arena_d = nc.dram_tensor('arena', [P, AB], u8, kind="ExternalInput")
    arena2_d = nc.dram_tensor('arena2', [32, 4 * 6400], u8, kind="ExternalInput")
    xT = {br: nc.dram_tensor(f'xT{br}', [cfg.NT, P, cfg.KC, 512], fp8,
                             kind="ExternalInput") for br in (1, 2)}
    scs_d = {br: nc.dram_tensor(
        f'scs{br}', [meta[f'npairs{br}'] // PAIRS_PER_CALL, P, 128 + GRP * BLK],
        u8, kind="ExternalInput") for br in (1, 2)}
    out_t = nc.dram_tensor('out', [P, 96], f32, kind="ExternalOutput")

    with tile.TileContext(nc) as tc:
        with tc.tile_pool(name="const", bufs=1) as cst, \
             tc.tile_pool(name="xt", bufs=5) as xtp, \
             tc.tile_pool(name="xwps", bufs=2, space="PSUM") as xwps, \
             tc.tile_pool(name="xwsb", bufs=3) as xwsb, \
             tc.tile_pool(name="scs", bufs=4) as scsp, \
             tc.tile_pool(name="gat", bufs=4) as gatp, \
             tc.tile_pool(name="blkps", bufs=2, space="PSUM") as blkps, \
             tc.tile_pool(name="hb", bufs=3) as hp, \
             tc.tile_pool(name="poolps", bufs=2, space="PSUM") as poolp, \
             tc.tile_pool(name="small", bufs=2) as smp, \
             tc.tile_pool(name="smallps", bufs=1, space="PSUM") as smps, \
             tc.tile_pool(name="dram", bufs=2, space="DRAM") as drp:

            # ---------------- constant arena (one DMA each)
            arena_t = cst.tile([P, AB], u8, tag='arena')
            nc.sync.dma_start(out=arena_t[:], in_=arena_d.ap())
            arena2_t = cst.tile([32, 4 * 6400], u8, tag='arena2')
            nc.sync.dma_start(out=arena2_t[:], in_=arena2_d.ap())

            def av(name, dtype):
                off, rows, key, shape, nb = offs[name]
                v = arena_t[0:rows, off:off + nb].bitcast(dtype)
                if len(shape) == 2:
                    v = v.rearrange("p (a b) -> p a b", a=shape[0])
                return v

            wg_v = {br: av(f'wg{br}', fp8) for br in (1, 2)}
            bg_v = {br: av(f'bg{br}', f32) for br in (1, 2)}
            mp_v = {br: av(f'mp{br}', bf16) for br in (1, 2)}
            wpf_v = {br: av(f'wpf{br}', f32) for br in (1, 2)}
            wm_v = {br: av(f'wm{br}', f32) for br in (1, 2)}
            bm_v = {br: av(f'bm{br}', f32) for br in (1, 2)}
            msc_v = {(br, sf, kind): av(f'{kind}_{sf}{br}', f32)
                     for br in (1, 2) for sf in 'sf'
                     for kind in ('scale', 'bias')}
            gmask_v = av('gmask', f32)

            id32 = cst.tile([32, 32], f32, tag='id32')
            make_identity(nc, id32[:])

            # ---------------- masif (both branches) -> two [64, B] f32 tiles
            masif_asm = {}
            for mi in (1, 2):
                frag = None
                for si, sf in enumerate('sf'):
                    toff = ((mi - 1) * 2 + si) * 6400
                    mv = arena2_t[:, toff:toff + 6400].bitcast(f32) \
                        .rearrange("p (c l) -> p c l", c=cfg.C)
                    red = smp.tile([32, cfg.LBS], f32, tag='masred')
                    nc.vector.tensor_reduce(
                        out=red[:], in_=mv.transpose([0, 2, 1]),
                        axis=mybir.AxisListType.X, op=OP.add)
                    act = smp.tile([32, cfg.LBS], f32, tag='masact')
                    nc.scalar.activation(
                        act[:], red[:], AF.Relu,
                        bias=msc_v[(mi, sf, 'bias')][:, 0:1],
                        scale=msc_v[(mi, sf, 'scale')][:, 0:1])
                    ws = smp.tile([32, cfg.WPB], f32, tag='masws')
                    nc.vector.tensor_reduce(
                        out=ws[:],
                        in_=act[:].rearrange("p (w l) -> p w l", l=cfg.LW),
                        axis=mybir.AxisListType.X, op=OP.add)
                    if frag is None:
                        frag = ws
                    else:
                        frag2 = smp.tile([32, cfg.WPB], f32, tag='masfrag')
                        nc.vector.tensor_add(out=frag2[:], in0=frag[:],
                                             in1=ws[:])
                        frag = frag2
                ps_t = smps.tile([cfg.WPB, 32], f32, space="PSUM", tag='spsA')
                nc.tensor.transpose(out=ps_t[:], in_=frag[:], identity=id32[:])
                fragT = smp.tile([cfg.WPB, 32], f32, tag='masfragT')
                nc.scalar.activation(fragT[:], ps_t[:], AF.Identity)
                fragTc = fragT[:].rearrange("k (lb g) -> k lb g", g=cfg.GPB)
                m_ps = smps.tile([64, cfg.GPB], f32, space="PSUM", tag='spsB')
                for lb in range(cfg.LB):
                    nc.tensor.matmul(
                        m_ps[:], lhsT=wm_v[mi][:, lb, :], rhs=fragTc[:, lb, :],
                        start=(lb == 0), stop=(lb == cfg.LB - 1))
                m_fm = smp.tile([64, cfg.GPB], f32, tag='masfm')
                nc.scalar.activation(m_fm[:], m_ps[:], AF.Identity,
                                     bias=bm_v[mi][:, 0:1])
                masm = cst.tile([64, cfg.B], f32, tag=f'masasm{mi}')
                nc.vector.tensor_tensor(
                    out=masm[:].rearrange("p (s g) -> p s g", g=cfg.GPB),
                    in0=m_fm[:, None, :].to_broadcast(
                        [64, N_CORES, cfg.GPB]),
                    in1=gmask_v.rearrange("p (s g) -> p s g", g=cfg.GPB),
                    op=OP.mult)
                masif_asm[mi] = masm

            nc.sync.dma_start(out=out_t.ap()[0:64, 64:96], in_=masif_asm[1][:])
            nc.sync.dma_start(out=out_t.ap()[64:128, 64:96],
                              in_=masif_asm[2][:])

            # ---------------- GCN branches
            for br in (1, 2):
                npairs = meta[f'npairs{br}']
                sched = meta[f'sched{br}']
                n_call = npairs // PAIRS_PER_CALL
                xw_dram = drp.tile([cfg.NPAD, cfg.FSL], fp8, tag='xwdram')

                # xw = x @ W slice (fp8 DoubleRow, x stationary)
                for nt in range(cfg.NT):
                    xt_t = xtp.tile([P, cfg.KC, 512], fp8, tag='xt')
                    nc.sync.dma_start(out=xt_t[:], in_=xT[br].ap()[nt])
                    xw_t = xwsb.tile([P, 4, cfg.FSL], fp8, tag='xwsb')
                    for sub in range(4):
                        ps = xwps.tile([P, cfg.FSL], f32, space="PSUM",
                                       tag='xwps')
                        for kp in range(cfg.KP):
                            nc.tensor.matmul(
                                ps[:],
                                lhsT=xt_t[:, 2 * kp:2 * kp + 2,
                                          sub * P:(sub + 1) * P],
                                rhs=wg_v[br][:, 2 * kp:2 * kp + 2, :],
                                start=(kp == 0), stop=(kp == cfg.KP - 1),
                                perf_mode=DR)
                        if sub % 2 == 0:
                            nc.vector.tensor_scalar_mul(
                                xw_t[:, sub, :], ps[:], 1.0)
                        else:
                            nc.scalar.activation(
                                xw_t[:, sub, :], ps[:], AF.Identity)
                    nc.sync.dma_start(
                        out=xw_dram[nt * 512:(nt + 1) * 512, :].rearrange(
                            "(s p) f -> p s f", p=P),
                        in_=xw_t[:])

                # scatter + transposed pool
                poolps = poolp.tile([P, 4, cfg.B], f32, space="PSUM",
                                    tag='poolps')
                pi = 0
                blk_ps = None
                for g in range(n_call):
                    scs_t = scsp.tile([P, 128 + GRP * BLK], u8, tag='scs')
                    nc.sync.dma_start(out=scs_t[:], in_=scs_d[br].ap()[g])
                    idx_v = scs_t[:, 0:128].bitcast(dt.int16)
                    s_v = scs_t[:, 128:128 + GRP * BLK].bitcast(fp8) \
                        .rearrange("p (c d) -> p c d", c=GRP)
                    gat_t = gatp.tile([P, GRP, cfg.FSL], fp8, tag='gat')
                    nc.gpsimd.dma_gather(
                        out_ap=gat_t[:], in_ap=xw_dram[:, :], idxs_ap=idx_v,
                        num_idxs=GRP * P, num_idxs_reg=GRP * P,
                        elem_size=cfg.FSL, queue_num=g % 2)
                    for i in range(PAIRS_PER_CALL):
                        j, st, sp = sched[pi]
                        if st:
                            blk_ps = blkps.tile([P, cfg.FSL], f32,
                                                space="PSUM", tag='blkps')
                        nc.tensor.matmul(
                            blk_ps[:],
                            lhsT=s_v[:, 2 * i:2 * i + 2, :],
                            rhs=gat_t[:, 2 * i:2 * i + 2, :],
                            start=st, stop=sp, perf_mode=DR)
                        if sp:
                            h_t = hp.tile([P, cfg.FSL], bf16, tag='h')
                            nc.vector.tensor_add(out=h_t[:], in0=blk_ps[:],
                                                 in1=bg_v[br])
                            nc.scalar.activation(h_t[:], h_t[:], AF.Lrelu,
                                                 alpha=0.01)
                            for c in range(4):
                                nc.tensor.matmul(
                                    poolps[:, c, :],
                                    lhsT=h_t[:, c * P:(c + 1) * P],
                                    rhs=mp_v[br][:, j, :],
                                    start=(j == 0), stop=(j == cfg.NBLK - 1),
                                    skip_group_check=True)
                        pi += 1

                # x_pre partial: [128, B] = W_pf^T @ pooledT
                pooled_sb = smp.tile([P, 4, cfg.B], f32, tag='pooled')
                nc.vector.tensor_scalar_mul(pooled_sb[:], poolps[:], 1.0)
                xpre_ps = smps.tile([P, cfg.B], f32, space="PSUM", tag='spsA')
                for c in range(4):
                    nc.tensor.matmul(xpre_ps[:], lhsT=wpf_v[br][:, c, :],
                                     rhs=pooled_sb[:, c, :],
                                     start=(c == 0), stop=(c == 3))
                xpre_sb = smp.tile([P, cfg.B], f32, tag='xpresb')
                nc.vector.tensor_scalar_mul(xpre_sb[:], xpre_ps[:], 1.0)
                nc.sync.dma_start(
                    out=out_t.ap()[:, (br - 1) * cfg.B:br * cfg.B],
                    in_=xpre_sb[:])

    nc.compile()
    return nc


# ---------------------------------------------------------------- entry
_CACHE = {}


def _run(inputs, cfg, trace=False, tmpdir=None):
    from concourse import bass_utils
    meta, in_maps = _preprocess(inputs, cfg)
    key = (cfg.N, cfg.F, meta['npairs1'], meta['npairs2'],
           tuple(x[0] for x in meta['sched1']),
           tuple(x[0] for x in meta['sched2']))
    if key not in _CACHE:
        _CACHE.clear()
        _CACHE[key] = _build(cfg, meta)
    nc = _CACHE[key]
    res = bass_utils.run_bass_kernel_spmd(
        nc, in_maps, core_ids=list(range(N_CORES)), trace=trace, tmpdir=tmpdir)
    xr = np.zeros((P, 96), np.float64)
    for i in range(N_CORES):
        xr += np.asarray(res.results[i]['out'], np.float32)
    out = _host_head(inputs, cfg, xr)
    return out, res


def _lrelu(v):
    return np.where(v > 0, v, 0.01 * v)


def _host_head(inputs, cfg, xr):
    """Unshard: sum of per-core partials -> tiny dense head (host)."""
    f32 = np.float32
    x1 = _lrelu(xr[:, 0:32] + np.asarray(inputs['b_pf1'], f32)[:, None])
    x2 = _lrelu(xr[:, 32:64] + np.asarray(inputs['b_pf2'], f32)[:, None])
    m1 = xr[0:64, 64:96]
    m2 = xr[64:128, 64:96]
    xcat = np.concatenate([x1, x2], 0)                       # [256, B]
    xc1 = _lrelu(np.asarray(inputs['W_fc1'], f32).T @ xcat
                 + np.asarray(inputs['b_fc1'], f32)[:, None])
    xc2 = _lrelu(np.asarray(inputs['W_fc2'], f32).T @ xc1
                 + np.asarray(inputs['b_fc2'], f32)[:, None])
    W_out = np.asarray(inputs['W_out'], f32)
    z = (W_out[0:64].T @ xc2 + W_out[64:128].T @ m1 + W_out[128:192].T @ m2
         + np.asarray(inputs['b_out'], f32)[:, None])
    return (1.0 / (1.0 + np.exp(-z))).T.astype(f32)         # [B, 1]


def kernel(**inputs) -> np.ndarray:
    cfg = _Cfg()
    out, _ = _run(inputs, cfg)
    return out
